# revision 19
# baseline (speedup 1.0000x reference)
"""Trainium2 Bass kernel: NeptuneTransformerEncoderLayer on 8 NeuronCores.

Sharding: batch(4) x seq-half(2) -> 8 cores, zero collectives.
Each core computes K/V for its batch's full 2048 tokens (10% redundant
FLOPs) and Q/attention/FFN for its own 1024 tokens.  The host permutes
each core's src so its query tokens are always rows [0:1024) -> one
uniform SPMD program.

Compute: fp16 operands on the PE (1 cycle/row), fp32 PSUM accumulation,
fp32 norm/softmax statistics.  All layout transposes ride the DMA xbar.
"""
import sys

for _p in ("/opt/trn_rl_repo", "/root/.axon_site/_ro/trn_rl_repo"):
    if _p not in sys.path:
        sys.path.insert(0, _p)

import numpy as np

import concourse.bass as bass
import concourse.mybir as mybir
import concourse.tile as tile
from concourse import bacc
from concourse import bass_utils

F16 = mybir.dt.float16
F32 = mybir.dt.float32
AF = mybir.ActivationFunctionType

P = 128
D = 1024            # d_model
DC = D // P         # 8 d-model chunks
NH = 16             # heads
HD = 64             # head dim
DFF = 4096
FC = DFF // P       # 32 ff chunks
S = 2048            # full sequence per batch
TQ = 1024           # query tokens per core
N_CORES = 8
EPS = 1e-5
BASE = 10000.0
ATT_DOWN = 1.0 / 64.0   # att_u eviction scale; undone by the 64/sum norm


def build_nc():
    nc = bacc.Bacc("TRN2", target_bir_lowering=False, debug=False,
                   num_devices=N_CORES)
    src = nc.dram_tensor("src", [S, D], F32, kind="ExternalInput")
    cos_t = nc.dram_tensor("cos_t", [P, S], F16, kind="ExternalInput")
    sin_t = nc.dram_tensor("sin_t", [P, S], F16, kind="ExternalInput")
    w_qkv = nc.dram_tensor("w_qkv", [3 * D, D], F32, kind="ExternalInput")
    w_out = nc.dram_tensor("w_out", [D, D], F32, kind="ExternalInput")
    w1 = nc.dram_tensor("w1", [DFF, D], F32, kind="ExternalInput")
    w2 = nc.dram_tensor("w2", [D, DFF], F32, kind="ExternalInput")
    w3 = nc.dram_tensor("w3", [DFF, D], F32, kind="ExternalInput")
    norm1_w = nc.dram_tensor("norm1_w", [D], F32, kind="ExternalInput")
    norm2_w = nc.dram_tensor("norm2_w", [D], F32, kind="ExternalInput")
    out = nc.dram_tensor("out", [TQ, D], F32, kind="ExternalOutput")

    with tile.TileContext(nc) as tc:
        emit(nc, tc, src, cos_t, sin_t, w_qkv, w_out, w1, w2, w3,
             norm1_w, norm2_w, out)
    nc.compile()
    return nc


def emit(nc, tc, src, cos_t, sin_t, w_qkv, w_out, w1, w2, w3,
         norm1_w, norm2_w, out):
    from contextlib import ExitStack

    ctx = ExitStack()
    with ctx:
        # pool groups with staged lifetimes (SBUF is 192KB/partition)
        g_xnt = ctx.enter_context(ExitStack())   # closed after phase B
        g_qkv = ctx.enter_context(ExitStack())   # closed after phase C
        g_att = ctx.enter_context(ExitStack())   # closed after phase D
        persist = ctx.enter_context(tc.tile_pool(name="persist", bufs=1))
        # p_xnt / p_att live on the right-side stack so their pop order is
        # independent of the left-side pools they interleave with
        p_xnt = g_xnt.enter_context(
            tc.tile_pool(name="p_xnt", bufs=1, side="right"))
        p_qkv = g_qkv.enter_context(tc.tile_pool(name="p_qkv", bufs=1))

        XNT = p_xnt.tile([P, DC, S], F16)        # x_norm1.T (d-major)
        C2 = p_xnt.tile([P, S], F16)
        S2 = p_xnt.tile([P, S], F16)
        QT = p_qkv.tile([P, DC, TQ], F16)        # roped q.T, parity-split rows
        KT = p_qkv.tile([P, DC, S], F16)         # roped k.T
        VA = p_qkv.tile([P, S // P, NH * 65], F16)  # v + ones col per head
        ones16 = persist.tile([1, HD], F16)
        nw1 = persist.tile([P, DC], F32)
        nw2 = persist.tile([P, DC], F32)
        eps_t = persist.tile([P, 1], F32)
        nc.vector.memset(eps_t[:], EPS)

        nc.sync.dma_start(C2[:], cos_t[:])
        nc.sync.dma_start(S2[:], sin_t[:])
        nc.vector.memset(ones16[:], 1.0)
        nc.sync.dma_start(nw1[:], norm1_w.ap().rearrange("(c p) -> p c", p=P))
        nc.sync.dma_start(nw2[:], norm2_w.ap().rearrange("(c p) -> p c", p=P))

        # ---------------- phase A: rmsnorm1 + transpose ----------------
        with tc.tile_pool(name="pha", bufs=4) as pha, \
             tc.tile_pool(name="pha_s", bufs=4) as pha_s:
            for ti in range(S // P):
                st = pha.tile([P, D], F32, tag="src_in")
                nc.sync.dma_start(st[:], src[ti * P:(ti + 1) * P, :])
                sq = pha.tile([P, D], F32, tag="sq")
                ssq = pha_s.tile([P, 1], F32, tag="ssq")
                nc.scalar.activation(sq[:], st[:], AF.Square, accum_out=ssq[:])
                rms = pha_s.tile([P, 1], F32, tag="rms")
                nc.scalar.activation(rms[:], ssq[:], AF.Sqrt,
                                     bias=eps_t[:], scale=1.0 / D)
                rinv = pha_s.tile([P, 1], F32, tag="rinv")
                nc.vector.reciprocal(rinv[:], rms[:])
                xn = pha.tile([P, D], F16, tag="xn")
                nc.vector.tensor_scalar_mul(xn[:], st[:], rinv[:])
                nc.sync.dma_start(XNT[:, :, ti * P:(ti + 1) * P], xn[:],
                                  transpose=True)
            # fold norm1_w into XNT (per-partition scalar along d)
            for c in range(DC):
                nc.vector.tensor_scalar_mul(XNT[:, c, :], XNT[:, c, :],
                                            nw1[:, c:c + 1])

        # ---------------- phase B: QKV ----------------
        # B1: Q & K, output transposed + roped, parity-split row layout
        with tc.tile_pool(name="phb", bufs=3) as phb, \
             tc.tile_pool(name="phb_ps", bufs=3, space="PSUM") as phb_ps:
            for j in range(16):          # o-chunks: 0-7 q, 8-15 k
                wraw = phb.tile([P, D], F32, tag="wraw")
                # permuted row load: partition p = 64*h + 32*par + jp
                # holds w_qkv row 128*j + 64*h + 2*jp + par
                rp = w_qkv.ap()[j * P:(j + 1) * P, :].rearrange(
                    "(h jp par) d -> h par jp d", h=2, jp=32, par=2)
                for hh in range(2):
                    for par in range(2):
                        nc.sync.dma_start(
                            wraw[hh * 64 + par * 32: hh * 64 + par * 32 + 32, :],
                            rp[hh, par])
                w16 = phb.tile([P, D], F16, tag="w16")
                nc.vector.tensor_copy(w16[:], wraw[:])
                wT = phb.tile([P, DC, P], F16, tag="wT")
                nc.sync.dma_start(wT[:], w16[:], transpose=True)
                T = TQ if j < 8 else S
                for ts in range(T // 512):
                    sl = slice(ts * 512, ts * 512 + 512)
                    pk = phb_ps.tile([P, 512], F32, tag="pk")
                    for c in range(DC):
                        nc.tensor.matmul(pk[:], wT[:, c, :], XNT[:, c, sl],
                                         start=(c == 0), stop=(c == DC - 1))
                    q16 = phb.tile([P, 512], F16, tag="q16")
                    nc.scalar.activation(q16[:], pk[:], AF.Copy)
                    aa = phb.tile([P, 512], F16, tag="aa")
                    nc.vector.tensor_mul(aa[:], q16[:], C2[:, sl])
                    pp = phb.tile([P, 512], F16, tag="pp")
                    nc.vector.tensor_mul(pp[:], q16[:], S2[:, sl])
                    bb = phb.tile([P, 512], F16, tag="bb")
                    for h0 in (0, 64):
                        nc.vector.tensor_copy(bb[h0:h0 + 32, :],
                                              pp[h0 + 32:h0 + 64, :])
                        nc.vector.tensor_copy(bb[h0 + 32:h0 + 64, :],
                                              pp[h0:h0 + 32, :])
                    dst = (QT[:, j, sl] if j < 8 else KT[:, j - 8, sl])
                    nc.vector.tensor_add(dst, aa[:], bb[:])

        # B2: V (token-major) + ones columns
        with tc.tile_pool(name="phv_w", bufs=1) as phv_w, \
             tc.tile_pool(name="phv", bufs=3) as phv, \
             tc.tile_pool(name="phv_ps", bufs=2, space="PSUM") as phv_ps:
            wvTs = []
            for j in range(8):
                wraw = phv.tile([P, D], F32, tag="wraw")
                nc.sync.dma_start(wraw[:], w_qkv[(16 + j) * P:(17 + j) * P, :])
                w16 = phv.tile([P, D], F16, tag="w16")
                nc.vector.tensor_copy(w16[:], wraw[:])
                wT = phv_w.tile([P, DC, P], F16, tag=f"wvT{j}")
                nc.sync.dma_start(wT[:], w16[:], transpose=True)
                wvTs.append(wT)
            va3 = VA.rearrange("p t (h c) -> p t h c", c=65)
            for ti in range(S // P):
                pv = phv_ps.tile([P, D], F32, tag="pv")
                for j in range(8):
                    for c in range(DC):
                        nc.tensor.matmul(pv[:, j * P:(j + 1) * P],
                                         XNT[:, c, ti * P:(ti + 1) * P],
                                         wvTs[j][:, c, :],
                                         start=(c == 0), stop=(c == DC - 1))
                nc.vector.memset(va3[:, ti, :, 64], 1.0)
                for half in range(2):
                    nc.scalar.activation(
                        va3[:, ti, half * 8:(half + 1) * 8, 0:64],
                        pv[:, half * 512:(half + 1) * 512].rearrange(
                            "p (h c) -> p h c", c=64),
                        AF.Copy)
        g_xnt.close()   # free XNT / C2 / S2
        p_att = g_att.enter_context(
            tc.tile_pool(name="p_att", bufs=1, side="right"))
        ATT = p_att.tile([P, DC, TQ], F16)       # normalized attn out .T

        # ---------------- phase C: attention ----------------
        with tc.tile_pool(name="phc", bufs=4) as phc, \
             tc.tile_pool(name="phc_st", bufs=2, space="PSUM") as phc_st, \
             tc.tile_pool(name="phc_pv", bufs=2, space="PSUM") as phc_pv:
            for h in range(NH):
                j, hb = h // 2, 64 * (h % 2)
                ppv = phc_pv.tile([65, TQ], F32, tag="ppv")
                for kc in range(S // P):
                    pst = phc_st.tile([P, TQ], F32, tag="pst")
                    for qs in range(2):
                        qsl = slice(qs * 512, qs * 512 + 512)
                        nc.tensor.matmul(pst[:, qsl],
                                         KT[hb:hb + 64, j, kc * P:(kc + 1) * P],
                                         QT[hb:hb + 64, j, qsl],
                                         start=True, stop=True)
                    pt16 = phc.tile([P, TQ], F16, tag="pt16")
                    nc.scalar.activation(pt16[:], pst[:], AF.Exp, scale=0.125)
                    for qs in range(2):
                        qsl = slice(qs * 512, qs * 512 + 512)
                        nc.tensor.matmul(ppv[:, qsl],
                                         VA[:, kc, 65 * h:65 * h + 65],
                                         pt16[:, qsl],
                                         start=(kc == 0), stop=(kc == S // P - 1))
                rec = phc.tile([1, TQ], F32, tag="rec")
                nc.vector.reciprocal(rec[:], ppv[64:65, :])
                r16 = phc.tile([1, TQ], F16, tag="r16")
                nc.vector.tensor_scalar_mul(r16[:], rec[:], 64.0)
                nrm = phc_st.tile([64, TQ], F32, tag="pst")
                for qs in range(2):
                    qsl = slice(qs * 512, qs * 512 + 512)
                    nc.tensor.matmul(nrm[:, qsl], ones16[:], r16[:, qsl],
                                     start=True, stop=True)
                att_u = phc.tile([64, TQ], F16, tag="att_u")
                nc.scalar.activation(att_u[:], ppv[0:64, :], AF.Copy,
                                     scale=ATT_DOWN)
                nc.vector.tensor_mul(ATT[hb:hb + 64, j, :], att_u[:], nrm[:])
        g_qkv.close()   # free QT / KT / VA

        # late big tiles, allocated in the space freed by QKV
        p_mid = ctx.enter_context(tc.tile_pool(name="p_mid", bufs=1))
        p_ht = ctx.enter_context(tc.tile_pool(name="p_ht", bufs=1))
        X2 = p_mid.tile([P, TQ // P, D], F32)    # post-attn residual (t-major)
        XN2T = p_mid.tile([P, DC, TQ], F16)      # x_norm2.T
        HT = p_ht.tile([P, FC, TQ], F16)         # swiglu hidden .T

        # ---------------- phase D: out_proj + residual + rmsnorm2 --------
        with tc.tile_pool(name="phd_w", bufs=1) as phd_w, \
             tc.tile_pool(name="phd", bufs=2) as phd, \
             tc.tile_pool(name="phd_s", bufs=4) as phd_s, \
             tc.tile_pool(name="phd_ps", bufs=2, space="PSUM") as phd_ps:
            woTs = []
            for j in range(8):
                wraw = phd.tile([P, D], F32, tag="wraw")
                nc.sync.dma_start(wraw[:], w_out[j * P:(j + 1) * P, :])
                w16 = phd.tile([P, D], F16, tag="w16")
                nc.vector.tensor_copy(w16[:], wraw[:])
                wT = phd_w.tile([P, DC, P], F16, tag=f"woT{j}")
                nc.sync.dma_start(wT[:], w16[:], transpose=True)
                woTs.append(wT)
            for tb in range(TQ // P):
                py = phd_ps.tile([P, D], F32, tag="py")
                for j in range(8):
                    for c in range(DC):
                        nc.tensor.matmul(py[:, j * P:(j + 1) * P],
                                         ATT[:, c, tb * P:(tb + 1) * P],
                                         woTs[j][:, c, :],
                                         start=(c == 0), stop=(c == DC - 1))
                srcq = phd.tile([P, D], F32, tag="srcq")
                nc.sync.dma_start(srcq[:], src[tb * P:(tb + 1) * P, :])
                nc.vector.tensor_add(X2[:, tb, :], py[:], srcq[:])
                sq = phd.tile([P, D], F32, tag="sq")
                ssq = phd_s.tile([P, 1], F32, tag="ssq")
                nc.scalar.activation(sq[:], X2[:, tb, :], AF.Square,
                                     accum_out=ssq[:])
                rms = phd_s.tile([P, 1], F32, tag="rms")
                nc.scalar.activation(rms[:], ssq[:], AF.Sqrt,
                                     bias=eps_t[:], scale=1.0 / D)
                rinv = phd_s.tile([P, 1], F32, tag="rinv")
                nc.vector.reciprocal(rinv[:], rms[:])
                xn2 = phd.tile([P, D], F16, tag="xn2")
                nc.vector.tensor_scalar_mul(xn2[:], X2[:, tb, :], rinv[:])
                nc.sync.dma_start(XN2T[:, :, tb * P:(tb + 1) * P], xn2[:],
                                  transpose=True)
            for c in range(DC):
                nc.vector.tensor_scalar_mul(XN2T[:, c, :], XN2T[:, c, :],
                                            nw2[:, c:c + 1])
        g_att.close()   # free ATT

        # ---------------- phase E: swiglu hidden ----------------
        with tc.tile_pool(name="phe", bufs=3) as phe, \
             tc.tile_pool(name="phe_ps", bufs=2, space="PSUM") as phe_ps:
            for fc in range(FC):
                w1raw = phe.tile([P, D], F32, tag="w1raw")
                nc.sync.dma_start(w1raw[:], w1[fc * P:(fc + 1) * P, :])
                w116 = phe.tile([P, D], F16, tag="w116")
                nc.vector.tensor_copy(w116[:], w1raw[:])
                w1T = phe.tile([P, DC, P], F16, tag="w1T")
                nc.sync.dma_start(w1T[:], w116[:], transpose=True)
                w3raw = phe.tile([P, D], F32, tag="w3raw")
                nc.sync.dma_start(w3raw[:], w3[fc * P:(fc + 1) * P, :])
                w316 = phe.tile([P, D], F16, tag="w316")
                nc.vector.tensor_copy(w316[:], w3raw[:])
                w3T = phe.tile([P, DC, P], F16, tag="w3T")
                nc.sync.dma_start(w3T[:], w316[:], transpose=True)
                pa = phe_ps.tile([P, TQ], F32, tag="pa")
                pb = phe_ps.tile([P, TQ], F32, tag="pb")
                for qs in range(2):
                    qsl = slice(qs * 512, qs * 512 + 512)
                    for c in range(DC):
                        nc.tensor.matmul(pa[:, qsl], w1T[:, c, :],
                                         XN2T[:, c, qsl],
                                         start=(c == 0), stop=(c == DC - 1))
                    for c in range(DC):
                        nc.tensor.matmul(pb[:, qsl], w3T[:, c, :],
                                         XN2T[:, c, qsl],
                                         start=(c == 0), stop=(c == DC - 1))
                sg = phe.tile([P, TQ], F16, tag="sg")
                nc.scalar.activation(sg[:], pa[:], AF.Sigmoid)
                sa = phe.tile([P, TQ], F16, tag="sa")
                nc.vector.tensor_mul(sa[:], sg[:], pa[:])
                nc.vector.tensor_mul(HT[:, fc, :], sa[:], pb[:])

        # ---------------- phase F: w2 + residual, output ----------------
        from concourse.masks import make_identity
        with tc.tile_pool(name="phf_c", bufs=1) as phf_c, \
             tc.tile_pool(name="phf", bufs=2) as phf, \
             tc.tile_pool(name="phf_y", bufs=1) as phf_y, \
             tc.tile_pool(name="phf_ps", bufs=2, space="PSUM") as phf_ps, \
             tc.tile_pool(name="phf_tp", bufs=4, space="PSUM") as phf_tp:
            ident = phf_c.tile([P, P], F32)
            make_identity(nc, ident)
            YT = phf_y.tile([P, DC, TQ], F32, tag="YT")
            HF = FC // 2
            for j in range(8):
                pz = phf_ps.tile([P, TQ], F32, tag="pz")
                for half in range(2):
                    w2raw = phf.tile([P, DFF // 2], F32, tag="w2raw")
                    nc.sync.dma_start(
                        w2raw[:],
                        w2[j * P:(j + 1) * P,
                           half * (DFF // 2):(half + 1) * (DFF // 2)])
                    w216 = phf.tile([P, DFF // 2], F16, tag="w216")
                    nc.vector.tensor_copy(w216[:], w2raw[:])
                    w2T = phf.tile([P, HF, P], F16, tag="w2T")
                    nc.sync.dma_start(w2T[:], w216[:], transpose=True)
                    for f in range(HF):
                        fc = half * HF + f
                        for qs in range(2):
                            qsl = slice(qs * 512, qs * 512 + 512)
                            nc.tensor.matmul(pz[:, qsl], w2T[:, f, :],
                                             HT[:, fc, qsl],
                                             start=(fc == 0),
                                             stop=(fc == FC - 1))
                nc.scalar.activation(YT[:, j, :], pz[:], AF.Copy)
            # transpose YT (o-major) back to token-major, add residual, store
            for tb in range(TQ // P):
                of = phf.tile([P, D], F32, tag="of")
                for j in range(8):
                    ptp = phf_tp.tile([P, P], F32, tag="ptp")
                    nc.tensor.transpose(ptp[:], YT[:, j, tb * P:(tb + 1) * P],
                                        ident[:])
                    nc.vector.tensor_add(of[:, j * P:(j + 1) * P],
                                         X2[:, tb, j * P:(j + 1) * P], ptp[:])
                nc.sync.dma_start(out[tb * P:(tb + 1) * P, :], of[:])


_NC_CACHE = None


def _get_nc():
    global _NC_CACHE
    if _NC_CACHE is None:
        _NC_CACHE = build_nc()
    return _NC_CACHE


def _host_tables(positions_b, axis_scale):
    """Build parity-split fp16 cos/sin tables (128, S) for one batch."""
    coord = positions_b * axis_scale[None, :]              # (S, 4)
    invf = BASE ** (-(np.arange(0, 16, 2, dtype=np.float32) / 16.0))  # (8,)
    ang = coord[:, :, None] * invf[None, None, :]          # (S, 4, 8)
    ang = ang.reshape(S, 32).T                             # (32, S): r = 8a + j
    cos64 = np.concatenate([np.cos(ang), np.cos(ang)], axis=0)   # (64, S)
    sin64 = np.concatenate([np.sin(ang), -np.sin(ang)], axis=0)  # (64, S)
    c128 = np.concatenate([cos64, cos64], axis=0).astype(np.float16)
    s128 = np.concatenate([sin64, sin64], axis=0).astype(np.float16)
    return c128, s128


def build_in_maps(inputs):
    src = np.asarray(inputs["src"], dtype=np.float32)
    positions = np.asarray(inputs["positions"], dtype=np.float32)
    axis_scale = np.asarray(inputs["axis_scale"], np.float32)
    weights = {k: np.asarray(inputs[k], np.float32)
               for k in ("w_qkv", "w_out", "w1", "w2", "w3",
                         "norm1_w", "norm2_w")}
    in_maps = []
    for c in range(N_CORES):
        b, h = c // 2, c % 2
        sp = src[b]
        pp = positions[b]
        if h == 1:  # own half first
            sp = np.concatenate([sp[TQ:], sp[:TQ]], axis=0)
            pp = np.concatenate([pp[TQ:], pp[:TQ]], axis=0)
        ct, st = _host_tables(pp, axis_scale)
        m = {"src": np.ascontiguousarray(sp), "cos_t": ct, "sin_t": st}
        m.update(weights)
        in_maps.append(m)
    return in_maps


def kernel(src, positions, w_qkv, w_out, norm1_w, norm2_w, w1, w2, w3,
           axis_scale):
    src = np.asarray(src, dtype=np.float32)
    B = src.shape[0]
    in_maps = build_in_maps(dict(
        src=src, positions=positions, w_qkv=w_qkv, w_out=w_out,
        norm1_w=norm1_w, norm2_w=norm2_w, w1=w1, w2=w2, w3=w3,
        axis_scale=axis_scale))
    nc = _get_nc()
    res = bass_utils.run_bass_kernel_spmd(nc, in_maps,
                                          core_ids=list(range(N_CORES)))
    outp = np.zeros((B, S, D), np.float32)
    for c in range(N_CORES):
        b, h = c // 2, c % 2
        outp[b, h * TQ:(h + 1) * TQ, :] = res.results[c]["out"]
    return outp


# revision 36
# speedup vs baseline: 1.0068x; 1.0068x over previous
"""Trainium2 Bass kernel: NeptuneTransformerEncoderLayer on 8 NeuronCores.

Sharding: batch(4) x seq-half(2) -> 8 cores, zero collectives.
Each core computes K/V for its batch's full 2048 tokens (10% redundant
FLOPs) and Q/attention/FFN for its own 1024 tokens.  The host permutes
each core's src so its query tokens are always rows [0:1024) -> one
uniform SPMD program.

Compute: fp16 operands on the PE (1 cycle/row), fp32 PSUM accumulation,
fp32 norm/softmax statistics.  All layout transposes ride the DMA xbar.
"""
import sys

for _p in ("/opt/trn_rl_repo", "/root/.axon_site/_ro/trn_rl_repo"):
    if _p not in sys.path:
        sys.path.insert(0, _p)

import numpy as np

import concourse.bass as bass
import concourse.mybir as mybir
import concourse.tile as tile
from concourse import bacc
from concourse import bass_utils

F16 = mybir.dt.float16
F32 = mybir.dt.float32
AF = mybir.ActivationFunctionType

P = 128
D = 1024            # d_model
DC = D // P         # 8 d-model chunks
NH = 16             # heads
HD = 64             # head dim
DFF = 4096
FC = DFF // P       # 32 ff chunks
S = 2048            # full sequence per batch
TQ = 1024           # query tokens per core
N_CORES = 8
EPS = 1e-5
BASE = 10000.0
ATT_DOWN = 1.0 / 64.0   # att_u eviction scale; undone by the 64/sum norm


def build_nc():
    nc = bacc.Bacc("TRN2", target_bir_lowering=False, debug=False,
                   num_devices=N_CORES)
    src = nc.dram_tensor("src", [S, D], F32, kind="ExternalInput")
    cos_t = nc.dram_tensor("cos_t", [P, S], F16, kind="ExternalInput")
    sin_t = nc.dram_tensor("sin_t", [P, S], F16, kind="ExternalInput")
    w_qkv = nc.dram_tensor("w_qkv", [3 * D, D], F32, kind="ExternalInput")
    w_out = nc.dram_tensor("w_out", [D, D], F32, kind="ExternalInput")
    w1 = nc.dram_tensor("w1", [DFF, D], F32, kind="ExternalInput")
    w2 = nc.dram_tensor("w2", [D, DFF], F32, kind="ExternalInput")
    w3 = nc.dram_tensor("w3", [DFF, D], F32, kind="ExternalInput")
    norm1_w = nc.dram_tensor("norm1_w", [D], F32, kind="ExternalInput")
    norm2_w = nc.dram_tensor("norm2_w", [D], F32, kind="ExternalInput")
    out = nc.dram_tensor("out", [TQ, D], F32, kind="ExternalOutput")

    with tile.TileContext(nc) as tc:
        emit(nc, tc, src, cos_t, sin_t, w_qkv, w_out, w1, w2, w3,
             norm1_w, norm2_w, out)
    nc.compile()
    return nc


def emit(nc, tc, src, cos_t, sin_t, w_qkv, w_out, w1, w2, w3,
         norm1_w, norm2_w, out):
    from contextlib import ExitStack

    ctx = ExitStack()
    with ctx:
        # pool groups with staged lifetimes (SBUF is 192KB/partition).
        # left stack: persist < p_att < p_qkv < phase pools
        # right stack: p_xnt, then p_mid/p_ht after it closes
        g_xnt = ctx.enter_context(ExitStack())   # closed after fused A/B/C
        g_qkv = ctx.enter_context(ExitStack())   # closed after fused A/B/C
        g_att = ctx.enter_context(ExitStack())   # closed after phase D
        persist = ctx.enter_context(tc.tile_pool(name="persist", bufs=1))
        p_att = g_att.enter_context(tc.tile_pool(name="p_att", bufs=1))
        p_xnt = g_xnt.enter_context(
            tc.tile_pool(name="p_xnt", bufs=1, side="right"))
        p_qkv = g_qkv.enter_context(tc.tile_pool(name="p_qkv", bufs=1))

        XNT = p_xnt.tile([P, DC, S], F16)        # x_norm1.T (d-major)
        C2 = p_xnt.tile([P, S], F16)
        S2 = p_xnt.tile([P, S], F16)
        QT = p_qkv.tile([P, DC, TQ], F16)        # roped q.T, parity-split rows
        KT = p_qkv.tile([P, DC, S], F16)         # roped k.T
        VA = p_qkv.tile([P, S // P, NH * 65], F16)  # v + ones col per head
        ATT = p_att.tile([P, DC, TQ], F16)       # normalized attn out .T
        eps_t = persist.tile([P, 1], F32)
        nc.vector.memset(eps_t[:], EPS)

        nc.sync.dma_start(C2[:], cos_t[:])
        nc.sync.dma_start(S2[:], sin_t[:])

        # ---- fused A/B/C: norm1+transpose, QKV+rope, V, attention ----
        # Emission order == engine stream order, arranged by data readiness:
        # src tiles, weight preps, then per head-pair (q, k, [V on pair 0],
        # attention) so the ACT-bound softmax overlaps the PE-bound QKV.
        with tc.tile_pool(name="pha", bufs=2) as pha, \
             tc.tile_pool(name="pha_s", bufs=4) as pha_s, \
             tc.tile_pool(name="phv_w", bufs=1) as phv_w, \
             tc.tile_pool(name="phb_w", bufs=2) as phb_w, \
             tc.tile_pool(name="phb", bufs=2) as phb, \
             tc.tile_pool(name="phc", bufs=2) as phc, \
             tc.tile_pool(name="ps_work", bufs=2, space="PSUM") as ps_work:

            def emit_a_ti(ti):
                st = pha.tile([P, D], F32, tag="src_in")
                nc.sync.dma_start(st[:], src[ti * P:(ti + 1) * P, :])
                ssq_a = pha_s.tile([P, 1], F32, tag="ssq_a")
                ssq_b = pha_s.tile([P, 1], F32, tag="ssq_b")
                for i, acc in ((0, ssq_a), (1, ssq_b)):
                    sq = ps_sq.tile([P, 512], F32, tag="sq")
                    nc.scalar.activation(sq[:], st[:, i * 512:(i + 1) * 512],
                                         AF.Square, accum_out=acc[:])
                ssq = pha_s.tile([P, 1], F32, tag="ssq")
                nc.vector.tensor_add(ssq[:], ssq_a[:], ssq_b[:])
                rms = pha_s.tile([P, 1], F32, tag="rms")
                nc.scalar.activation(rms[:], ssq[:], AF.Sqrt,
                                     bias=eps_t[:], scale=1.0 / D)
                rinv = pha_s.tile([P, 1], F32, tag="rinv")
                nc.vector.reciprocal(rinv[:], rms[:])
                xn = pha.tile([P, D], F16, tag="xn")
                nc.vector.tensor_scalar_mul(xn[:], st[:], rinv[:])
                nc.sync.dma_start(XNT[:, :, ti * P:(ti + 1) * P], xn[:],
                                  transpose=True)

            def prep_qk(j):
                # permuted row load (cast f32->f16 in SWDGE): partition
                # p = 64*h + 32*par + jp holds w_qkv row 128*j+64*h+2*jp+par
                w16 = phb_w.tile([P, D], F16, tag="w16")
                rp = w_qkv.ap()[j * P:(j + 1) * P, :].rearrange(
                    "(h jp par) d -> h par jp d", h=2, jp=32, par=2)
                for hh in range(2):
                    nc.gpsimd.dma_start(w16[hh * 64:(hh + 1) * 64, :], rp[hh])
                wT = phb_w.tile([P, DC, P], F16, tag="wT")
                nc.sync.dma_start(wT[:], w16[:], transpose=True)
                return wT

            def prep_v(j):
                w16 = phb_w.tile([P, D], F16, tag="w16")
                nc.gpsimd.dma_start(w16[:], w_qkv[(16 + j) * P:(17 + j) * P, :])
                wT = phv_w.tile([P, DC, P], F16, tag=f"wvT{j}")
                nc.sync.dma_start(wT[:], w16[:], transpose=True)
                return wT

            def compute_qk(j, wT):
                T = TQ if j < 8 else S
                for ts in range(T // 512):
                    sl = slice(ts * 512, ts * 512 + 512)
                    pk = ps_work.tile([P, 512], F32, tag="work")
                    for c in range(DC):
                        nc.tensor.matmul(pk[:], wT[:, c, :], XNT[:, c, sl],
                                         start=(c == 0), stop=(c == DC - 1))
                    aa = phb.tile([P, 512], F16, tag="aa")
                    nc.vector.tensor_mul(aa[:], pk[:], C2[:, sl])
                    pp = phb.tile([P, 512], F16, tag="pp")
                    nc.vector.tensor_mul(pp[:], pk[:], S2[:, sl])
                    bb = phb.tile([P, 512], F16, tag="bb")
                    for h0 in (0, 64):
                        nc.vector.tensor_copy(bb[h0:h0 + 32, :],
                                              pp[h0 + 32:h0 + 64, :])
                        nc.vector.tensor_copy(bb[h0 + 32:h0 + 64, :],
                                              pp[h0:h0 + 32, :])
                    dst = (QT[:, j, sl] if j < 8 else KT[:, j - 8, sl])
                    nc.vector.tensor_add(dst, aa[:], bb[:])

            va3 = VA.rearrange("p t (h c) -> p t h c", c=65)

            def emit_v_ti(ti, wvTs):
                for half in range(2):
                    pvh = ps_work.tile([P, 512], F32, tag="work")
                    for j4 in range(4):
                        j = half * 4 + j4
                        for c in range(DC):
                            nc.tensor.matmul(
                                pvh[:, j4 * P:(j4 + 1) * P],
                                XNT[:, c, ti * P:(ti + 1) * P],
                                wvTs[j][:, c, :],
                                start=(c == 0), stop=(c == DC - 1))
                    hs = slice(half * 8, (half + 1) * 8)
                    nc.vector.memset(va3[:, ti, hs, 64], 1.0)
                    nc.vector.tensor_copy(
                        va3[:, ti, hs, 0:64],
                        pvh.rearrange("p (h c) -> p h c", c=64))

            def attend(h, ps_st, ps_pv):
                j, hb = h // 2, 64 * (h % 2)
                ppv = ps_pv.tile([65, TQ], F32, tag="ppv")
                for kc in range(S // P):
                    pst = ps_st.tile([P, TQ], F32, tag="pst")
                    for qs in range(2):
                        qsl = slice(qs * 512, qs * 512 + 512)
                        nc.tensor.matmul(
                            pst[:, qsl],
                            KT[hb:hb + 64, j, kc * P:(kc + 1) * P],
                            QT[hb:hb + 64, j, qsl],
                            start=True, stop=True)
                    pt16 = phc.tile([P, TQ], F16, tag="pt16")
                    nc.scalar.activation(pt16[:], pst[:], AF.Exp, scale=0.125)
                    for qs in range(2):
                        qsl = slice(qs * 512, qs * 512 + 512)
                        nc.tensor.matmul(ppv[:, qsl],
                                         VA[:, kc, 65 * h:65 * h + 65],
                                         pt16[:, qsl],
                                         start=(kc == 0),
                                         stop=(kc == S // P - 1))
                rec = phc.tile([1, TQ], F32, tag="rec")
                nc.vector.reciprocal(rec[:], ppv[64:65, :])
                r16 = phc.tile([1, TQ], F16, tag="r16")
                nc.vector.tensor_scalar_mul(r16[:], rec[:], 64.0)
                nrmS = phc.tile([64, TQ], F16, tag="nrmS")
                nc.gpsimd.partition_broadcast(nrmS[:], r16[:])
                att_u = phc.tile([64, TQ], F16, tag="att_u")
                nc.scalar.activation(att_u[:], ppv[0:64, :], AF.Copy,
                                     scale=ATT_DOWN)
                nc.vector.tensor_mul(ATT[hb:hb + 64, j, :], att_u[:], nrmS[:])

            with tc.tile_pool(name="ps_sq", bufs=2, space="PSUM") as ps_sq:
                for ti in range(4):
                    emit_a_ti(ti)
                pend = {0: prep_qk(0), 8: prep_qk(8)}
                for ti in range(4, S // P):
                    emit_a_ti(ti)
                wvTs = [prep_v(j) for j in range(8)]

            with tc.tile_pool(name="ps_st", bufs=2, space="PSUM") as ps_st, \
                 tc.tile_pool(name="ps_pv", bufs=1, space="PSUM") as ps_pv:
                for hp in range(8):
                    for jj in (hp, 8 + hp):
                        wT = pend.pop(jj, None)
                        if wT is None:
                            wT = prep_qk(jj)
                        compute_qk(jj, wT)
                    if hp == 0:
                        for ti in range(S // P):
                            emit_v_ti(ti, wvTs)
                    if hp < 7:
                        pend[hp + 1] = prep_qk(hp + 1)
                        pend[9 + hp] = prep_qk(9 + hp)
                    attend(2 * hp, ps_st, ps_pv)
                    attend(2 * hp + 1, ps_st, ps_pv)
        g_qkv.close()   # free QT / KT / VA
        g_xnt.close()   # free XNT / C2 / S2

        # late big tiles, allocated in the space freed by QKV
        p_mid = ctx.enter_context(tc.tile_pool(name="p_mid", bufs=1, side="right"))
        p_ht = ctx.enter_context(tc.tile_pool(name="p_ht", bufs=1, side="right"))
        X2 = p_mid.tile([P, TQ // P, D], F32)    # post-attn residual (t-major)
        XN2T = p_mid.tile([P, DC, TQ], F16)      # x_norm2.T
        HT = p_ht.tile([P, FC, TQ], F16)         # swiglu hidden .T

        # ---------------- phase D: out_proj + residual + rmsnorm2 --------
        with tc.tile_pool(name="phd_w", bufs=1) as phd_w, \
             tc.tile_pool(name="phd", bufs=3) as phd, \
             tc.tile_pool(name="phd_s", bufs=4) as phd_s, \
             tc.tile_pool(name="phd_ps", bufs=2, space="PSUM") as phd_ps:
            woTs = []
            for j in range(8):
                w16 = phd.tile([P, D], F16, tag="w16")
                nc.gpsimd.dma_start(w16[:], w_out[j * P:(j + 1) * P, :])
                wT = phd_w.tile([P, DC, P], F16, tag=f"woT{j}")
                nc.sync.dma_start(wT[:], w16[:], transpose=True)
                woTs.append(wT)
            for tb in range(TQ // P):
                py = phd_ps.tile([P, D], F32, tag="py")
                for j in range(8):
                    for c in range(DC):
                        nc.tensor.matmul(py[:, j * P:(j + 1) * P],
                                         ATT[:, c, tb * P:(tb + 1) * P],
                                         woTs[j][:, c, :],
                                         start=(c == 0), stop=(c == DC - 1))
                srcq = phd.tile([P, D], F32, tag="srcq")
                nc.sync.dma_start(srcq[:], src[tb * P:(tb + 1) * P, :])
                nc.vector.tensor_add(X2[:, tb, :], py[:], srcq[:])
                sq = phd.tile([P, D], F32, tag="sq")
                ssq = phd_s.tile([P, 1], F32, tag="ssq")
                nc.scalar.activation(sq[:], X2[:, tb, :], AF.Square,
                                     accum_out=ssq[:])
                rms = phd_s.tile([P, 1], F32, tag="rms")
                nc.scalar.activation(rms[:], ssq[:], AF.Sqrt,
                                     bias=eps_t[:], scale=1.0 / D)
                rinv = phd_s.tile([P, 1], F32, tag="rinv")
                nc.vector.reciprocal(rinv[:], rms[:])
                xn2 = phd.tile([P, D], F16, tag="xn2")
                nc.vector.tensor_scalar_mul(xn2[:], X2[:, tb, :], rinv[:])
                nc.sync.dma_start(XN2T[:, :, tb * P:(tb + 1) * P], xn2[:],
                                  transpose=True)
            # norm2_w is folded into w1/w3 on the host
        g_att.close()   # free ATT

        # ---------------- phase E: swiglu hidden ----------------
        with tc.tile_pool(name="phe", bufs=3) as phe, \
             tc.tile_pool(name="phe_ps", bufs=2, space="PSUM") as phe_ps:
            for fc in range(FC):
                w116 = phe.tile([P, D], F16, tag="w116")
                nc.gpsimd.dma_start(w116[:], w1[fc * P:(fc + 1) * P, :])
                w1T = phe.tile([P, DC, P], F16, tag="w1T")
                nc.sync.dma_start(w1T[:], w116[:], transpose=True)
                w316 = phe.tile([P, D], F16, tag="w316")
                nc.gpsimd.dma_start(w316[:], w3[fc * P:(fc + 1) * P, :])
                w3T = phe.tile([P, DC, P], F16, tag="w3T")
                nc.sync.dma_start(w3T[:], w316[:], transpose=True)
                pa = phe_ps.tile([P, TQ], F32, tag="pa")
                pb = phe_ps.tile([P, TQ], F32, tag="pb")
                for qs in range(2):
                    qsl = slice(qs * 512, qs * 512 + 512)
                    for c in range(DC):
                        nc.tensor.matmul(pa[:, qsl], w1T[:, c, :],
                                         XN2T[:, c, qsl],
                                         start=(c == 0), stop=(c == DC - 1))
                    for c in range(DC):
                        nc.tensor.matmul(pb[:, qsl], w3T[:, c, :],
                                         XN2T[:, c, qsl],
                                         start=(c == 0), stop=(c == DC - 1))
                sg = phe.tile([P, TQ], F16, tag="sg")
                nc.scalar.activation(sg[:], pa[:], AF.Sigmoid)
                sa = phe.tile([P, TQ], F16, tag="sa")
                nc.vector.tensor_mul(sa[:], sg[:], pa[:])
                nc.vector.tensor_mul(HT[:, fc, :], sa[:], pb[:])

        # ---------------- phase F: w2 + residual, output ----------------
        from concourse.masks import make_identity
        with tc.tile_pool(name="phf_c", bufs=1) as phf_c, \
             tc.tile_pool(name="phf", bufs=2) as phf, \
             tc.tile_pool(name="phf_y", bufs=1) as phf_y, \
             tc.tile_pool(name="phf_ps", bufs=2, space="PSUM") as phf_ps, \
             tc.tile_pool(name="phf_tp", bufs=4, space="PSUM") as phf_tp:
            ident = phf_c.tile([P, P], F32)
            make_identity(nc, ident)
            YT = phf_y.tile([P, DC, TQ], F32, tag="YT")
            HF = FC // 2
            for j in range(8):
                pz = phf_ps.tile([P, TQ], F32, tag="pz")
                for half in range(2):
                    w216 = phf.tile([P, DFF // 2], F16, tag="w216")
                    nc.gpsimd.dma_start(
                        w216[:],
                        w2[j * P:(j + 1) * P,
                           half * (DFF // 2):(half + 1) * (DFF // 2)])
                    w2T = phf.tile([P, HF, P], F16, tag="w2T")
                    nc.sync.dma_start(w2T[:], w216[:], transpose=True)
                    for f in range(HF):
                        fc = half * HF + f
                        for qs in range(2):
                            qsl = slice(qs * 512, qs * 512 + 512)
                            nc.tensor.matmul(pz[:, qsl], w2T[:, f, :],
                                             HT[:, fc, qsl],
                                             start=(fc == 0),
                                             stop=(fc == FC - 1))
                nc.scalar.activation(YT[:, j, :], pz[:], AF.Copy)
            # transpose YT (o-major) back to token-major, add residual, store
            for tb in range(TQ // P):
                of = phf.tile([P, D], F32, tag="of")
                for j in range(8):
                    ptp = phf_tp.tile([P, P], F32, tag="ptp")
                    nc.tensor.transpose(ptp[:], YT[:, j, tb * P:(tb + 1) * P],
                                        ident[:])
                    nc.vector.tensor_add(of[:, j * P:(j + 1) * P],
                                         X2[:, tb, j * P:(j + 1) * P], ptp[:])
                nc.sync.dma_start(out[tb * P:(tb + 1) * P, :], of[:])


_NC_CACHE = None


def _get_nc():
    global _NC_CACHE
    if _NC_CACHE is None:
        _NC_CACHE = build_nc()
    return _NC_CACHE


def _host_tables(positions_b, axis_scale):
    """Build parity-split fp16 cos/sin tables (128, S) for one batch."""
    coord = positions_b * axis_scale[None, :]              # (S, 4)
    invf = BASE ** (-(np.arange(0, 16, 2, dtype=np.float32) / 16.0))  # (8,)
    ang = coord[:, :, None] * invf[None, None, :]          # (S, 4, 8)
    ang = ang.reshape(S, 32).T                             # (32, S): r = 8a + j
    cos64 = np.concatenate([np.cos(ang), np.cos(ang)], axis=0)   # (64, S)
    sin64 = np.concatenate([np.sin(ang), -np.sin(ang)], axis=0)  # (64, S)
    c128 = np.concatenate([cos64, cos64], axis=0).astype(np.float16)
    s128 = np.concatenate([sin64, sin64], axis=0).astype(np.float16)
    return c128, s128


def build_in_maps(inputs):
    src = np.asarray(inputs["src"], dtype=np.float32)
    positions = np.asarray(inputs["positions"], dtype=np.float32)
    axis_scale = np.asarray(inputs["axis_scale"], np.float32)
    n1 = np.asarray(inputs["norm1_w"], np.float32)
    n2 = np.asarray(inputs["norm2_w"], np.float32)
    weights = {
        # rmsnorm gains are folded into the consuming projections
        "w_qkv": np.asarray(inputs["w_qkv"], np.float32) * n1[None, :],
        "w_out": np.asarray(inputs["w_out"], np.float32),
        "w1": np.asarray(inputs["w1"], np.float32) * n2[None, :],
        "w3": np.asarray(inputs["w3"], np.float32) * n2[None, :],
        "w2": np.asarray(inputs["w2"], np.float32),
        "norm1_w": n1, "norm2_w": n2,
    }
    in_maps = []
    for c in range(N_CORES):
        b, h = c // 2, c % 2
        sp = src[b]
        pp = positions[b]
        if h == 1:  # own half first
            sp = np.concatenate([sp[TQ:], sp[:TQ]], axis=0)
            pp = np.concatenate([pp[TQ:], pp[:TQ]], axis=0)
        ct, st = _host_tables(pp, axis_scale)
        m = {"src": np.ascontiguousarray(sp), "cos_t": ct, "sin_t": st}
        m.update(weights)
        in_maps.append(m)
    return in_maps


def kernel(src, positions, w_qkv, w_out, norm1_w, norm2_w, w1, w2, w3,
           axis_scale):
    src = np.asarray(src, dtype=np.float32)
    B = src.shape[0]
    in_maps = build_in_maps(dict(
        src=src, positions=positions, w_qkv=w_qkv, w_out=w_out,
        norm1_w=norm1_w, norm2_w=norm2_w, w1=w1, w2=w2, w3=w3,
        axis_scale=axis_scale))
    nc = _get_nc()
    res = bass_utils.run_bass_kernel_spmd(nc, in_maps,
                                          core_ids=list(range(N_CORES)))
    outp = np.zeros((B, S, D), np.float32)
    for c in range(N_CORES):
        b, h = c // 2, c % 2
        outp[b, h * TQ:(h + 1) * TQ, :] = res.results[c]["out"]
    return outp


# revision 40
# speedup vs baseline: 78.2800x; 77.7508x over previous
"""Trainium2 Bass kernel: NeptuneTransformerEncoderLayer on 8 NeuronCores.

Sharding: batch(4) x seq-half(2) -> 8 cores, zero collectives.
Each core computes K/V for its batch's full 2048 tokens (10% redundant
FLOPs) and Q/attention/FFN for its own 1024 tokens.  The host permutes
each core's src so its query tokens are always rows [0:1024) -> one
uniform SPMD program.

Compute: fp16 operands on the PE (1 cycle/row), fp32 PSUM accumulation,
fp32 norm/softmax statistics.  All layout transposes ride the DMA xbar.
"""
import sys

for _p in ("/opt/trn_rl_repo", "/root/.axon_site/_ro/trn_rl_repo"):
    if _p not in sys.path:
        sys.path.insert(0, _p)

import numpy as np

import concourse.bass as bass
import concourse.mybir as mybir
import concourse.tile as tile
from concourse import bacc
from concourse import bass_utils

F16 = mybir.dt.float16
F32 = mybir.dt.float32
AF = mybir.ActivationFunctionType

P = 128
D = 1024            # d_model
DC = D // P         # 8 d-model chunks
NH = 16             # heads
HD = 64             # head dim
DFF = 4096
FC = DFF // P       # 32 ff chunks
S = 2048            # full sequence per batch
TQ = 1024           # query tokens per core
N_CORES = 8
EPS = 1e-5
BASE = 10000.0
ATT_DOWN = 1.0 / 64.0   # att_u eviction scale; undone by the 64/sum norm


def build_nc():
    nc = bacc.Bacc("TRN2", target_bir_lowering=False, debug=False,
                   num_devices=N_CORES)
    src = nc.dram_tensor("src", [S, D], F32, kind="ExternalInput")
    cos_t = nc.dram_tensor("cos_t", [P, S], F16, kind="ExternalInput")
    sin_t = nc.dram_tensor("sin_t", [P, S], F16, kind="ExternalInput")
    w_qkv = nc.dram_tensor("w_qkv", [3 * D, D], F32, kind="ExternalInput")
    w_out = nc.dram_tensor("w_out", [D, D], F32, kind="ExternalInput")
    w1 = nc.dram_tensor("w1", [DFF, D], F32, kind="ExternalInput")
    w2 = nc.dram_tensor("w2", [D, DFF], F32, kind="ExternalInput")
    w3 = nc.dram_tensor("w3", [DFF, D], F32, kind="ExternalInput")
    norm1_w = nc.dram_tensor("norm1_w", [D], F32, kind="ExternalInput")
    norm2_w = nc.dram_tensor("norm2_w", [D], F32, kind="ExternalInput")
    out = nc.dram_tensor("out", [TQ, D], F32, kind="ExternalOutput")

    with tile.TileContext(nc) as tc:
        emit(nc, tc, src, cos_t, sin_t, w_qkv, w_out, w1, w2, w3,
             norm1_w, norm2_w, out)
    nc.compile()
    return nc


def emit(nc, tc, src, cos_t, sin_t, w_qkv, w_out, w1, w2, w3,
         norm1_w, norm2_w, out):
    from contextlib import ExitStack

    ctx = ExitStack()
    with ctx:
        # pool groups with staged lifetimes (SBUF is 192KB/partition).
        # left stack: persist < p_att < p_qkv < phase pools
        # right stack: p_xnt, then p_mid/p_ht after it closes
        g_xnt = ctx.enter_context(ExitStack())   # closed after fused A/B/C
        g_qkv = ctx.enter_context(ExitStack())   # closed after fused A/B/C
        g_att = ctx.enter_context(ExitStack())   # closed after phase D
        persist = ctx.enter_context(tc.tile_pool(name="persist", bufs=1))
        p_att = g_att.enter_context(tc.tile_pool(name="p_att", bufs=1))
        p_xnt = g_xnt.enter_context(
            tc.tile_pool(name="p_xnt", bufs=1, side="right"))
        p_qkv = g_qkv.enter_context(tc.tile_pool(name="p_qkv", bufs=1))

        XNT = p_xnt.tile([P, DC, S], F16)        # x_norm1.T (d-major)
        C2 = p_xnt.tile([P, S], F16)
        S2 = p_xnt.tile([P, S], F16)
        QT = p_qkv.tile([P, DC, TQ], F16)        # roped q.T, parity-split rows
        KT = p_qkv.tile([P, DC, S], F16)         # roped k.T
        VA = p_qkv.tile([P, S // P, NH * 65], F16)  # v + ones col per head
        ATT = p_att.tile([P, DC, TQ], F16)       # normalized attn out .T
        eps_t = persist.tile([P, 1], F32)
        nc.vector.memset(eps_t[:], EPS)

        nc.sync.dma_start(C2[:], cos_t[:])
        nc.sync.dma_start(S2[:], sin_t[:])

        # ---- fused A/B/C: norm1+transpose, QKV+rope, V, attention ----
        # Emission order == engine stream order, arranged by data readiness:
        # src tiles, weight preps, then per head-pair (q, k, [V on pair 0],
        # attention) so the ACT-bound softmax overlaps the PE-bound QKV.
        with tc.tile_pool(name="pha", bufs=2) as pha, \
             tc.tile_pool(name="pha_s", bufs=4) as pha_s, \
             tc.tile_pool(name="phv_w", bufs=1) as phv_w, \
             tc.tile_pool(name="phb_w", bufs=2) as phb_w, \
             tc.tile_pool(name="phb", bufs=2) as phb, \
             tc.tile_pool(name="phc", bufs=2) as phc, \
             tc.tile_pool(name="ps_work", bufs=2, space="PSUM") as ps_work:

            def emit_a_ti(ti):
                st = pha.tile([P, D], F32, tag="src_in")
                nc.sync.dma_start(st[:], src[ti * P:(ti + 1) * P, :])
                ssq_a = pha_s.tile([P, 1], F32, tag="ssq_a")
                ssq_b = pha_s.tile([P, 1], F32, tag="ssq_b")
                for i, acc in ((0, ssq_a), (1, ssq_b)):
                    sq = ps_sq.tile([P, 512], F32, tag="sq")
                    nc.scalar.activation(sq[:], st[:, i * 512:(i + 1) * 512],
                                         AF.Square, accum_out=acc[:])
                ssq = pha_s.tile([P, 1], F32, tag="ssq")
                nc.vector.tensor_add(ssq[:], ssq_a[:], ssq_b[:])
                rms = pha_s.tile([P, 1], F32, tag="rms")
                nc.scalar.activation(rms[:], ssq[:], AF.Sqrt,
                                     bias=eps_t[:], scale=1.0 / D)
                rinv = pha_s.tile([P, 1], F32, tag="rinv")
                nc.vector.reciprocal(rinv[:], rms[:])
                xn = pha.tile([P, D], F16, tag="xn")
                nc.vector.tensor_scalar_mul(xn[:], st[:], rinv[:])
                nc.sync.dma_start(XNT[:, :, ti * P:(ti + 1) * P], xn[:],
                                  transpose=True)

            def prep_qk(j):
                # permuted row load (cast f32->f16 in SWDGE): partition
                # p = 64*h + 32*par + jp holds w_qkv row 128*j+64*h+2*jp+par
                w16 = phb_w.tile([P, D], F16, tag="w16")
                rp = w_qkv.ap()[j * P:(j + 1) * P, :].rearrange(
                    "(h jp par) d -> h par jp d", h=2, jp=32, par=2)
                for hh in range(2):
                    nc.gpsimd.dma_start(w16[hh * 64:(hh + 1) * 64, :], rp[hh])
                wT = phb_w.tile([P, DC, P], F16, tag="wT")
                nc.sync.dma_start(wT[:], w16[:], transpose=True)
                return wT

            def prep_v(j):
                w16 = phb_w.tile([P, D], F16, tag="w16")
                nc.gpsimd.dma_start(w16[:], w_qkv[(16 + j) * P:(17 + j) * P, :])
                wT = phv_w.tile([P, DC, P], F16, tag=f"wvT{j}")
                nc.sync.dma_start(wT[:], w16[:], transpose=True)
                return wT

            def qk_slice(j, wT, ts):
                sl = slice(ts * 512, ts * 512 + 512)
                pk = ps_work.tile([P, 512], F32, tag="work")
                for c in range(DC):
                    nc.tensor.matmul(pk[:], wT[:, c, :], XNT[:, c, sl],
                                     start=(c == 0), stop=(c == DC - 1))
                aa = phb.tile([P, 512], F16, tag="aa")
                nc.vector.tensor_mul(aa[:], pk[:], C2[:, sl])
                pp = phb.tile([P, 512], F16, tag="pp")
                nc.vector.tensor_mul(pp[:], pk[:], S2[:, sl])
                bb = phb.tile([P, 512], F16, tag="bb")
                for h0 in (0, 64):
                    nc.vector.tensor_copy(bb[h0:h0 + 32, :],
                                          pp[h0 + 32:h0 + 64, :])
                    nc.vector.tensor_copy(bb[h0 + 32:h0 + 64, :],
                                          pp[h0:h0 + 32, :])
                dst = (QT[:, j, sl] if j < 8 else KT[:, j - 8, sl])
                nc.vector.tensor_add(dst, aa[:], bb[:])

            def compute_qk(j, wT):
                for ts in range((TQ if j < 8 else S) // 512):
                    qk_slice(j, wT, ts)

            va3 = VA.rearrange("p t (h c) -> p t h c", c=65)

            def emit_v_ti(ti, wvTs):
                for half in range(2):
                    pvh = ps_work.tile([P, 512], F32, tag="work")
                    for j4 in range(4):
                        j = half * 4 + j4
                        for c in range(DC):
                            nc.tensor.matmul(
                                pvh[:, j4 * P:(j4 + 1) * P],
                                XNT[:, c, ti * P:(ti + 1) * P],
                                wvTs[j][:, c, :],
                                start=(c == 0), stop=(c == DC - 1))
                    hs = slice(half * 8, (half + 1) * 8)
                    nc.vector.memset(va3[:, ti, hs, 64], 1.0)
                    nc.vector.tensor_copy(
                        va3[:, ti, hs, 0:64],
                        pvh.rearrange("p (h c) -> p h c", c=64))

            def attend(h, ps_st, ps_pv):
                j, hb = h // 2, 64 * (h % 2)
                ppv = ps_pv.tile([65, TQ], F32, tag="ppv")
                for kc in range(S // P):
                    pst = ps_st.tile([P, TQ], F32, tag="pst")
                    for qs in range(2):
                        qsl = slice(qs * 512, qs * 512 + 512)
                        nc.tensor.matmul(
                            pst[:, qsl],
                            KT[hb:hb + 64, j, kc * P:(kc + 1) * P],
                            QT[hb:hb + 64, j, qsl],
                            start=True, stop=True)
                    pt16 = phc.tile([P, TQ], F16, tag="pt16")
                    nc.scalar.activation(pt16[:], pst[:], AF.Exp, scale=0.125)
                    for qs in range(2):
                        qsl = slice(qs * 512, qs * 512 + 512)
                        nc.tensor.matmul(ppv[:, qsl],
                                         VA[:, kc, 65 * h:65 * h + 65],
                                         pt16[:, qsl],
                                         start=(kc == 0),
                                         stop=(kc == S // P - 1))
                rec = phc.tile([1, TQ], F32, tag="rec")
                nc.vector.reciprocal(rec[:], ppv[64:65, :])
                r16 = phc.tile([1, TQ], F16, tag="r16")
                nc.vector.tensor_scalar_mul(r16[:], rec[:], 64.0)
                nrmS = phc.tile([64, TQ], F16, tag="nrmS")
                nc.gpsimd.partition_broadcast(nrmS[:], r16[:])
                att_u = phc.tile([64, TQ], F16, tag="att_u")
                nc.scalar.activation(att_u[:], ppv[0:64, :], AF.Copy,
                                     scale=ATT_DOWN)
                nc.vector.tensor_mul(ATT[hb:hb + 64, j, :], att_u[:], nrmS[:])

            with tc.tile_pool(name="ps_sq", bufs=2, space="PSUM") as ps_sq:
                for ti in range(4):
                    emit_a_ti(ti)
                wt0 = prep_qk(0)
                wv0 = prep_v(0)
                wv1 = prep_v(1)
                wt8 = prep_qk(8)
                for ti in range(4, S // P):
                    emit_a_ti(ti)
                wvTs = [wv0, wv1] + [prep_v(j) for j in range(2, 8)]

            with tc.tile_pool(name="ps_st", bufs=2, space="PSUM") as ps_st, \
                 tc.tile_pool(name="ps_pv", bufs=1, space="PSUM") as ps_pv:
                # pair 0: weave q/k slices with V rows in src-tile readiness
                # order so the PE is never waiting on late src tiles
                V = lambda ti: emit_v_ti(ti, wvTs)
                V(0)
                qk_slice(0, wt0, 0); V(1); V(2)
                qk_slice(0, wt0, 1); V(3); V(4)
                qk_slice(8, wt8, 0); V(5); V(6)
                qk_slice(8, wt8, 1); V(7); V(8)
                qk_slice(8, wt8, 2); V(9); V(10); V(11)
                qk_slice(8, wt8, 3)
                for ti in range(12, S // P):
                    V(ti)
                pend = {1: prep_qk(1), 9: prep_qk(9)}
                attend(0, ps_st, ps_pv)
                attend(1, ps_st, ps_pv)
                for hp in range(1, 8):
                    for jj in (hp, 8 + hp):
                        wT = pend.pop(jj, None)
                        if wT is None:
                            wT = prep_qk(jj)
                        compute_qk(jj, wT)
                    if hp < 7:
                        pend[hp + 1] = prep_qk(hp + 1)
                        pend[9 + hp] = prep_qk(9 + hp)
                    attend(2 * hp, ps_st, ps_pv)
                    attend(2 * hp + 1, ps_st, ps_pv)
        g_qkv.close()   # free QT / KT / VA
        g_xnt.close()   # free XNT / C2 / S2

        # late big tiles, allocated in the space freed by QKV
        p_mid = ctx.enter_context(tc.tile_pool(name="p_mid", bufs=1, side="right"))
        p_ht = ctx.enter_context(tc.tile_pool(name="p_ht", bufs=1, side="right"))
        X2 = p_mid.tile([P, TQ // P, D], F32)    # post-attn residual (t-major)
        XN2T = p_mid.tile([P, DC, TQ], F16)      # x_norm2.T
        HT = p_ht.tile([P, FC, TQ], F16)         # swiglu hidden .T

        # ---------------- phase D: out_proj + residual + rmsnorm2 --------
        with tc.tile_pool(name="phd_w", bufs=1) as phd_w, \
             tc.tile_pool(name="phd", bufs=3) as phd, \
             tc.tile_pool(name="phd_s", bufs=4) as phd_s, \
             tc.tile_pool(name="phd_ps", bufs=2, space="PSUM") as phd_ps:
            woTs = []
            for j in range(8):
                w16 = phd.tile([P, D], F16, tag="w16")
                nc.gpsimd.dma_start(w16[:], w_out[j * P:(j + 1) * P, :])
                wT = phd_w.tile([P, DC, P], F16, tag=f"woT{j}")
                nc.sync.dma_start(wT[:], w16[:], transpose=True)
                woTs.append(wT)
            for tb in range(TQ // P):
                py = phd_ps.tile([P, D], F32, tag="py")
                for j in range(8):
                    for c in range(DC):
                        nc.tensor.matmul(py[:, j * P:(j + 1) * P],
                                         ATT[:, c, tb * P:(tb + 1) * P],
                                         woTs[j][:, c, :],
                                         start=(c == 0), stop=(c == DC - 1))
                srcq = phd.tile([P, D], F32, tag="srcq")
                nc.sync.dma_start(srcq[:], src[tb * P:(tb + 1) * P, :])
                nc.vector.tensor_add(X2[:, tb, :], py[:], srcq[:])
                sq = phd.tile([P, D], F32, tag="sq")
                ssq = phd_s.tile([P, 1], F32, tag="ssq")
                nc.scalar.activation(sq[:], X2[:, tb, :], AF.Square,
                                     accum_out=ssq[:])
                rms = phd_s.tile([P, 1], F32, tag="rms")
                nc.scalar.activation(rms[:], ssq[:], AF.Sqrt,
                                     bias=eps_t[:], scale=1.0 / D)
                rinv = phd_s.tile([P, 1], F32, tag="rinv")
                nc.vector.reciprocal(rinv[:], rms[:])
                xn2 = phd.tile([P, D], F16, tag="xn2")
                nc.vector.tensor_scalar_mul(xn2[:], X2[:, tb, :], rinv[:])
                nc.sync.dma_start(XN2T[:, :, tb * P:(tb + 1) * P], xn2[:],
                                  transpose=True)
            # norm2_w is folded into w1/w3 on the host
        g_att.close()   # free ATT

        # ---------------- phase E: swiglu hidden ----------------
        with tc.tile_pool(name="phe", bufs=3) as phe, \
             tc.tile_pool(name="phe_ps", bufs=2, space="PSUM") as phe_ps:
            for fc in range(FC):
                w116 = phe.tile([P, D], F16, tag="w116")
                nc.gpsimd.dma_start(w116[:], w1[fc * P:(fc + 1) * P, :])
                w1T = phe.tile([P, DC, P], F16, tag="w1T")
                nc.sync.dma_start(w1T[:], w116[:], transpose=True)
                w316 = phe.tile([P, D], F16, tag="w316")
                nc.gpsimd.dma_start(w316[:], w3[fc * P:(fc + 1) * P, :])
                w3T = phe.tile([P, DC, P], F16, tag="w3T")
                nc.sync.dma_start(w3T[:], w316[:], transpose=True)
                pa = phe_ps.tile([P, TQ], F32, tag="pa")
                pb = phe_ps.tile([P, TQ], F32, tag="pb")
                for qs in range(2):
                    qsl = slice(qs * 512, qs * 512 + 512)
                    for c in range(DC):
                        nc.tensor.matmul(pa[:, qsl], w1T[:, c, :],
                                         XN2T[:, c, qsl],
                                         start=(c == 0), stop=(c == DC - 1))
                    for c in range(DC):
                        nc.tensor.matmul(pb[:, qsl], w3T[:, c, :],
                                         XN2T[:, c, qsl],
                                         start=(c == 0), stop=(c == DC - 1))
                sg = phe.tile([P, TQ], F16, tag="sg")
                nc.scalar.activation(sg[:], pa[:], AF.Sigmoid)
                sa = phe.tile([P, TQ], F16, tag="sa")
                nc.vector.tensor_mul(sa[:], sg[:], pa[:])
                nc.vector.tensor_mul(HT[:, fc, :], sa[:], pb[:])

        # ---------------- phase F: w2 + residual, output ----------------
        from concourse.masks import make_identity
        with tc.tile_pool(name="phf_c", bufs=1) as phf_c, \
             tc.tile_pool(name="phf", bufs=2) as phf, \
             tc.tile_pool(name="phf_y", bufs=1) as phf_y, \
             tc.tile_pool(name="phf_ps", bufs=2, space="PSUM") as phf_ps, \
             tc.tile_pool(name="phf_tp", bufs=4, space="PSUM") as phf_tp:
            ident = phf_c.tile([P, P], F32)
            make_identity(nc, ident)
            HF = FC // 2
            ofs = []
            for tb in range(TQ // P):
                of = phf_y.tile([P, D], F32, tag=f"of{tb}")
                ofs.append(of)
            for j in range(8):
                pz = phf_ps.tile([P, TQ], F32, tag="pz")
                for half in range(2):
                    w216 = phf.tile([P, DFF // 2], F16, tag="w216")
                    nc.gpsimd.dma_start(
                        w216[:],
                        w2[j * P:(j + 1) * P,
                           half * (DFF // 2):(half + 1) * (DFF // 2)])
                    w2T = phf.tile([P, HF, P], F16, tag="w2T")
                    nc.sync.dma_start(w2T[:], w216[:], transpose=True)
                    for f in range(HF):
                        fc = half * HF + f
                        for qs in range(2):
                            qsl = slice(qs * 512, qs * 512 + 512)
                            nc.tensor.matmul(pz[:, qsl], w2T[:, f, :],
                                             HT[:, fc, qsl],
                                             start=(fc == 0),
                                             stop=(fc == FC - 1))
                yj = phf.tile([P, TQ], F32, tag="yj")
                nc.scalar.activation(yj[:], pz[:], AF.Copy)
                # immediately fold this j-column back to token-major
                for tb in range(TQ // P):
                    ptp = phf_tp.tile([P, P], F32, tag="ptp")
                    nc.tensor.transpose(ptp[:], yj[:, tb * P:(tb + 1) * P],
                                        ident[:])
                    nc.vector.tensor_add(ofs[tb][:, j * P:(j + 1) * P],
                                         X2[:, tb, j * P:(j + 1) * P], ptp[:])
            for tb in range(TQ // P):
                nc.sync.dma_start(out[tb * P:(tb + 1) * P, :], ofs[tb][:])


_NC_CACHE = None


def _get_nc():
    global _NC_CACHE
    if _NC_CACHE is None:
        _NC_CACHE = build_nc()
    return _NC_CACHE


def _host_tables(positions_b, axis_scale):
    """Build parity-split fp16 cos/sin tables (128, S) for one batch."""
    coord = positions_b * axis_scale[None, :]              # (S, 4)
    invf = BASE ** (-(np.arange(0, 16, 2, dtype=np.float32) / 16.0))  # (8,)
    ang = coord[:, :, None] * invf[None, None, :]          # (S, 4, 8)
    ang = ang.reshape(S, 32).T                             # (32, S): r = 8a + j
    cos64 = np.concatenate([np.cos(ang), np.cos(ang)], axis=0)   # (64, S)
    sin64 = np.concatenate([np.sin(ang), -np.sin(ang)], axis=0)  # (64, S)
    c128 = np.concatenate([cos64, cos64], axis=0).astype(np.float16)
    s128 = np.concatenate([sin64, sin64], axis=0).astype(np.float16)
    return c128, s128


def build_in_maps(inputs):
    src = np.asarray(inputs["src"], dtype=np.float32)
    positions = np.asarray(inputs["positions"], dtype=np.float32)
    axis_scale = np.asarray(inputs["axis_scale"], np.float32)
    n1 = np.asarray(inputs["norm1_w"], np.float32)
    n2 = np.asarray(inputs["norm2_w"], np.float32)
    weights = {
        # rmsnorm gains are folded into the consuming projections
        "w_qkv": np.asarray(inputs["w_qkv"], np.float32) * n1[None, :],
        "w_out": np.asarray(inputs["w_out"], np.float32),
        "w1": np.asarray(inputs["w1"], np.float32) * n2[None, :],
        "w3": np.asarray(inputs["w3"], np.float32) * n2[None, :],
        "w2": np.asarray(inputs["w2"], np.float32),
        "norm1_w": n1, "norm2_w": n2,
    }
    in_maps = []
    for c in range(N_CORES):
        b, h = c // 2, c % 2
        sp = src[b]
        pp = positions[b]
        if h == 1:  # own half first
            sp = np.concatenate([sp[TQ:], sp[:TQ]], axis=0)
            pp = np.concatenate([pp[TQ:], pp[:TQ]], axis=0)
        ct, st = _host_tables(pp, axis_scale)
        m = {"src": np.ascontiguousarray(sp), "cos_t": ct, "sin_t": st}
        m.update(weights)
        in_maps.append(m)
    return in_maps


def kernel(src, positions, w_qkv, w_out, norm1_w, norm2_w, w1, w2, w3,
           axis_scale):
    src = np.asarray(src, dtype=np.float32)
    B = src.shape[0]
    in_maps = build_in_maps(dict(
        src=src, positions=positions, w_qkv=w_qkv, w_out=w_out,
        norm1_w=norm1_w, norm2_w=norm2_w, w1=w1, w2=w2, w3=w3,
        axis_scale=axis_scale))
    nc = _get_nc()
    res = bass_utils.run_bass_kernel_spmd(nc, in_maps,
                                          core_ids=list(range(N_CORES)))
    outp = np.zeros((B, S, D), np.float32)
    for c in range(N_CORES):
        b, h = c // 2, c % 2
        outp[b, h * TQ:(h + 1) * TQ, :] = res.results[c]["out"]
    return outp
